# revision 64
# baseline (speedup 1.0000x reference)
"""Self-contained Trainium2 Bass kernel for a 12-head attention layer.

Problem: x[4,2048,768] -> attention(QKV projections, softmax, context),
NUM_HEADS=12, SIZE_PER_HEAD=64, additive mask from mask[4,2048].

Sharding over 8 NeuronCores: core c handles batch b=c//2 and head-group
hg=c%2 (6 heads, 384 feature columns).  Everything is local per core:
no collectives.

Design (v12 -- head-pair row-tiled scores, cross-segment pipeline):
  - Projections Q^T,K^T [384,2048] and V^T feature-major: full K=128/M=128
    bf16 matmuls.  V^T is XBAR-transpose-DMA'd (contiguous scratch, then
    DVE copy) into token-major vp [128, 16T, 6x128]: per head 64 V-dims,
    a ones column (-> softmax denominator via the ctx matmul) and zero
    padding to 128.
  - Scores matmuls have K=64 (half the PE rows).  Heads are processed in
    PAIRS: the even head's K/Q live at SBUF partitions 0:64, the odd
    head's at 64:128, so the two scores matmuls auto-derive PE row-tiles
    (0,0)/(64,0) (64x128 mode, tiles T0/T8) and execute CONCURRENTLY --
    2x scores throughput vs serial (2nd MM of each pair traces at ~3ns).
    Each pair-slot produces one [128, 2*fw] psum tile (even head cols
    0:fw -> bank0, odd head at col-offset 512 -> bank1; disjoint banks,
    required for row-tile concurrency).
  - Softmax exp on ACT over the whole pair tile ([128,1024] ~1.01us);
    ACT and PE co-pace (~85% busy each): exp work 6x2048x2048 per core
    ~= 164us streaming at 1 elem/cycle/lane @1.2GHz; PE ~= 216us busy
    incl ~100ns tiling-mode-switch drains (64x128 scores <-> 128x128
    ctx/proj) that the list scheduler makes unavoidable.  1/sqrt(64)
    folds into the activation scale; mask enters as a per-partition
    bias (all-ones mask specializes to bias 0).
  - Ctx matmuls (K=128, full array; already 1 col/cycle -- do NOT
    row-tile-split full-K contractions, measured +18us) accumulate per
    head into separate [128,512] psum banks across the 16 T-tiles.
  - Segments: (pair, 512-wide F-chunk) = 3x4 = 12, t-tiles in groups of
    2 (one 64-mode scores block, then one 128-mode ctx block + fills).
    Scores psum double-buffered [128,1024]x2 (4 banks), ctx pair
    2x[128,512] (2 banks), proj psum [128,512]x2 (2 banks) = 8 PSUM
    banks exactly.  The ctx stream trails scores/exp by >=2 exps and
    flows ACROSS segment boundaries (pend carries segment state; a
    segment's drain+normalize emits when its last ctx pops), so ACT
    never waits for a segment flush.  Projections interleave between
    groups as fills per a deadline-driven schedule.
  - Normalize per segment: DMA-gather denominators -> DVE reciprocal ->
    DMA scatter -> GpSimd partition_broadcast -> DVE multiply -> DMA out
    (GpSimd is ~4x slower than DVE per element -- keep bulk elementwise
    off it); the last segment uses PE ones-broadcast instead of the
    GpSimd hop (shorter serial tail).
  - PE warmup (8 matmuls on a zeroed tile, 64x128 mode) releases the HAM
    clock throttle during the input-DMA window; x chunk 0 is the first
    DMA (per-queue issue order is the only DMA priority control; packets
    only start flowing ~8.5us in, so the prefix is DMA-gated).
  - Rejected by measurement: fp8 DoubleRow ctx (error >> 2e-2 budget),
    Schraudolph bit-trick exp on DVE (each offloaded tile stalls the
    2-slot scores-psum rotation by ~ what it saves on ACT), ko-split
    prefix DMAs (descriptor issue serializes at ~0.6us each on the Sync
    queue).

Output per core: ctx^T [384,2048] f32; host transposes to [4,2048,768].
"""

import numpy as np
import ml_dtypes

B, S, D = 4, 2048, 768
H, DH = 12, 64
HL = 6          # heads per core
DL = HL * DH    # 384 feature columns per core
NCORES = 8
P = 128
KO = 6          # full k-subtiles of the 768 contraction
NT = S // P     # 16 T-tiles
FW = 512        # F chunk per head per segment
NPAIR = 3       # head pairs per core

# token layout inside a T-tile, set by the V transpose DMA semantics:
# vp[p, mt, :] holds token mt*P + p  (natural; probe-verified)
TOK_INTERLEAVED = False

_CACHE = {}


def _build(with_bias=False, ones_mask=True):
    import concourse.mybir as mybir
    import concourse.tile as tile
    from concourse import bacc

    dt = mybir.dt
    Exp = mybir.ActivationFunctionType.Exp

    nc = bacc.Bacc("TRN2", target_bir_lowering=False, debug=False,
                   num_devices=NCORES)

    xT = nc.dram_tensor("xT", [D, S], dt.bfloat16, kind="ExternalInput")
    wq = nc.dram_tensor("wq", [D + 1, DL], dt.bfloat16, kind="ExternalInput")
    wk = nc.dram_tensor("wk", [D + 1, DL], dt.bfloat16, kind="ExternalInput")
    wv = nc.dram_tensor("wv", [D + 1, DL], dt.bfloat16, kind="ExternalInput")
    adder = nc.dram_tensor("adder", [P, NT], dt.float32, kind="ExternalInput")
    out = nc.dram_tensor("out", [DL, S], dt.float32, kind="ExternalOutput")

    KE = KO + 1 if with_bias else KO

    with tile.TileContext(nc) as tc:
        with (
            tc.tile_pool(name="persist", bufs=1) as sb,
            tc.tile_pool(name="etp", bufs=16) as etp,
            tc.tile_pool(name="vfp", bufs=3) as vfp,
            tc.tile_pool(name="fin", bufs=2) as fin,
            tc.tile_pool(name="ps_s", bufs=2, space="PSUM") as ps_s,
            tc.tile_pool(name="ps_c", bufs=2, space="PSUM") as ps_c,
            tc.tile_pool(name="ps_p", bufs=2, space="PSUM") as ps_p,
        ):
            # ---------------- input DMA (priority order) ----------------
            wqs = sb.tile([P, KO + 1, DL], dt.bfloat16, tag="wqs")
            wks = sb.tile([P, KO + 1, DL], dt.bfloat16, tag="wks")
            wvs = sb.tile([P, KO + 1, DL], dt.bfloat16, tag="wvs")
            xTs = sb.tile([P, KO + 1, S], dt.bfloat16, tag="xTs")
            adder_sb = sb.tile([P, NT], dt.float32, tag="adder")

            def dma_w(w_dram, w_sb, c0, c1):
                nc.sync.dma_start(
                    w_sb[:, 0:KO, c0:c1],
                    w_dram.ap()[0:D, c0:c1].rearrange(
                        "(ko p) m -> p ko m", p=P))
                if with_bias:
                    nc.sync.dma_start(w_sb[0:1, KO, c0:c1],
                                      w_dram.ap()[D:D + 1, c0:c1])

            def dma_x(s0, s1):
                nc.sync.dma_start(
                    xTs[:, 0:KO, s0:s1],
                    xT.ap()[:, s0:s1].rearrange("(ko p) s -> p ko s", p=P))

            # x chunk 0 first: it is the biggest prefix-critical transfer
            # and per-queue issue order is the only DMA priority control
            dma_x(0, 512)
            dma_w(wq, wqs, 0, P)
            dma_w(wk, wks, 0, P)
            dma_w(wv, wvs, 0, P)
            dma_x(512, 1024)
            dma_x(1024, S)
            dma_w(wv, wvs, P, DL)
            dma_w(wq, wqs, P, DL)
            dma_w(wk, wks, P, DL)
            if not ones_mask:
                nc.sync.dma_start(adder_sb[:], adder.ap())
            if with_bias:
                nc.gpsimd.memset(xTs[0:1, KO, :], 1.0)

            warm = sb.tile([P, 512], dt.bfloat16, tag="warm")
            nc.gpsimd.memset(warm[:], 0.0)

            # persistent projection outputs
            qtb = sb.tile([P, 3, S], dt.bfloat16, tag="qtb")
            ktb = sb.tile([P, 3, S], dt.bfloat16, tag="ktb")
            vp = sb.tile([P, NT, HL * P], dt.bfloat16, tag="vp")
            for h in range(HL):
                nc.gpsimd.memset(vp[:, :, h * P + DH:h * P + DH + 1], 1.0)
                nc.gpsimd.memset(vp[:, :, h * P + DH + 1:(h + 1) * P], 0.0)

            # ---------------- projection helpers ----------------
            # one 512-wide proj chunk: psum <- W_mtile^T @ x_chunk.
            # (K=128 matmuls already stream at 1 col/cycle -- row-tile
            # splitting a full-K contraction doubles columns for zero gain,
            # measured +18us PE; keep the plain form.)
            def proj_mm(w_sb, mt, c):
                pt = ps_p.tile([P, 512], dt.float32, tag="p", name="pt")
                for k in range(KE):
                    lhsT = (w_sb[:, k, mt * P:(mt + 1) * P] if k < KO
                            else w_sb[0:1, k, mt * P:(mt + 1) * P])
                    rhs = (xTs[:, k, c * 512:(c + 1) * 512] if k < KO
                           else xTs[0:1, k, c * 512:(c + 1) * 512])
                    nc.tensor.matmul(pt[:], lhsT, rhs,
                                     start=(k == 0), stop=(k == KE - 1))
                return pt

            def qk_chunk(which, mt, c):
                w_sb = wqs if which == "q" else wks
                pt = proj_mm(w_sb, mt, c)
                dstt = qtb if which == "q" else ktb
                nc.vector.tensor_copy(
                    dstt[:, mt, c * 512:(c + 1) * 512], pt[:])

            vf_stage = {}

            def vf_chunk(mt, c):
                # feature-major V' proj chunk; after chunk 3, XBAR-transpose
                # both heads of this m-tile into token-major vp.
                pt = proj_mm(wvs, mt, c)
                if c == 0:
                    vf_stage[mt] = vfp.tile([P, S], dt.bfloat16,
                                            tag="vf", name="vf")
                vt = vf_stage[mt]
                nc.vector.tensor_copy(vt[:, c * 512:(c + 1) * 512], pt[:])
                if c == 3:
                    for hip in range(2):
                        h = 2 * mt + hip
                        # XBAR transpose needs a contiguous destination
                        # (strided dst slices produce wrong output on HW);
                        # bounce through a scratch tile, DVE copies into
                        # the strided vp layout.
                        vph = vfp.tile([P, NT, DH], dt.bfloat16,
                                       tag="vph", name="vph")
                        nc.sync.dma_start_transpose(
                            vph[:], vt[hip * DH:(hip + 1) * DH, :])
                        nc.vector.tensor_copy(
                            vp[:, :, h * P:h * P + DH], vph[:])
                    del vf_stage[mt]

            # ---------------- warmup (HAM un-throttle during DMA) --------
            # runs on uninitialized SBUF (results never read; psum slot is
            # reclaimed via start=True) so the PE starts at t~0 with no
            # upstream deps.
            wexp = sb.tile([P, 1], dt.bfloat16, tag="wexp")
            nc.scalar.activation(wexp[:], warm[:, 0:1], Exp)
            ones1 = sb.tile([1, DH], dt.bfloat16, tag="ones1")
            nc.gpsimd.memset(ones1[:], 1.0)
            # warmup in (64,128) mode -- matches the scores/proj tiling so
            # the first real matmul doesn't pay a mode-switch drain
            wpt = ps_p.tile([P, 512], dt.float32, tag="p", name="wpt")
            for wi in range(8):
                nc.tensor.matmul(wpt[:], warm[0:DH, 0:P], warm[0:DH, :],
                                 start=(wi == 0), stop=(wi == 7))

            # ---------------- prefix projections ----------------
            # k first: the first scores matmul loads ktb as weights
            qk_chunk("k", 0, 0)
            qk_chunk("q", 0, 0)

            # ---------------- fill schedule ----------------
            # per-segment lists; ordering constraints:
            #   q(pi,c) before seg (pi,c) starts; k(pi,c) before that
            #   pair's scores hit ti=4c; vf(pi,*)+transpose before the
            #   pair's first ctx.
            from collections import deque
            F = lambda fn, *a: (lambda: fn(*a))
            seg_fills = [
                # seg0 (p0,c0): k0 c1/c2/c3 by ti 4/8/12; vf0 early for ctx
                [F(vf_chunk, 0, 0), F(vf_chunk, 0, 1), F(qk_chunk, "k", 0, 1),
                 F(vf_chunk, 0, 2), F(vf_chunk, 0, 3), F(qk_chunk, "k", 0, 2),
                 F(qk_chunk, "q", 0, 1), F(qk_chunk, "k", 0, 3)],
                # segs 1-10: at most 3 fills each so pops stay on the
                # proven {g1,g3,g5} slots -- a 4th fill pops at g7 and
                # lands right in front of the next segment's first scores
                # pair, stalling the ACT exp stream at the boundary
                # seg1 (p0,c1)
                [F(qk_chunk, "q", 0, 2), F(qk_chunk, "k", 1, 0),
                 F(qk_chunk, "k", 1, 1)],
                # seg2 (p0,c2)
                [F(qk_chunk, "q", 0, 3), F(vf_chunk, 1, 0),
                 F(vf_chunk, 1, 1)],
                # seg3 (p0,c3)
                [F(vf_chunk, 1, 2), F(vf_chunk, 1, 3), F(qk_chunk, "q", 1, 0)],
                # seg4 (p1,c0): k(1,2)/k(1,3) pop at g1/g3, ahead of their
                # own-segment uses at g4/g6 (PE in-order)
                [F(qk_chunk, "k", 1, 2), F(qk_chunk, "k", 1, 3),
                 F(qk_chunk, "q", 1, 1)],
                # seg5 (p1,c1)
                [F(qk_chunk, "q", 1, 2), F(qk_chunk, "k", 2, 0),
                 F(qk_chunk, "k", 2, 1)],
                # seg6 (p1,c2)
                [F(qk_chunk, "q", 1, 3), F(vf_chunk, 2, 0),
                 F(vf_chunk, 2, 1)],
                # seg7 (p1,c3)
                [F(vf_chunk, 2, 2), F(vf_chunk, 2, 3), F(qk_chunk, "q", 2, 0)],
                # seg8 (p2,c0): k(2,2)/k(2,3) at g1/g3 before g4/g6 uses
                [F(qk_chunk, "k", 2, 2), F(qk_chunk, "k", 2, 3),
                 F(qk_chunk, "q", 2, 1)],
                [F(qk_chunk, "q", 2, 2)],
                [F(qk_chunk, "q", 2, 3)],
                [],
            ]

            # ---------------- attention segments ----------------
            # pair pi heads (2pi, 2pi+1); even head K/Q at partitions 0:64,
            # odd head at 64:128 -> concurrent row-tiled scores matmuls.
            def emit_scores(pi, f0, fw, ti):
                sp = ps_s.tile([P, 2 * FW], dt.float32, tag="s", name="sp")
                ks = ktb[:, pi, ti * P:(ti + 1) * P]
                qs = qtb[:, pi, f0:f0 + fw]
                nc.tensor.matmul(sp[:, 0:fw], ks[0:DH, :], qs[0:DH, :],
                                 start=True, stop=True)
                nc.tensor.matmul(sp[:, FW:FW + fw], ks[DH:P, :], qs[DH:P, :],
                                 start=True, stop=True)
                return sp

            def emit_exp(sp, fw, ti):
                et = etp.tile([P, 2 * FW], dt.bfloat16, tag="et", name="et")
                bias = 0.0 if ones_mask else adder_sb[:, ti:ti + 1]
                if fw == FW:
                    nc.scalar.activation(et[:], sp[:], Exp,
                                         bias=bias, scale=0.125)
                else:
                    nc.scalar.activation(et[:, 0:fw], sp[:, 0:fw], Exp,
                                         bias=bias, scale=0.125)
                    nc.scalar.activation(et[:, FW:FW + fw], sp[:, FW:FW + fw],
                                         Exp, bias=bias, scale=0.125)
                return et

            def emit_ctx(cp0, cp1, pi, fw, ti, et):
                h0, h1 = 2 * pi, 2 * pi + 1
                nc.tensor.matmul(
                    cp0[:, 0:fw], vp[:, ti, h0 * P:(h0 + 1) * P],
                    et[:, 0:fw], start=(ti == 0), stop=(ti == NT - 1))
                nc.tensor.matmul(
                    cp1[:, 0:fw], vp[:, ti, h1 * P:(h1 + 1) * P],
                    et[:, FW:FW + fw], start=(ti == 0), stop=(ti == NT - 1))

            def normalize(pi, f0, fw, ctxs_t):
                # ctxs_t [DH+1, 2*fw]: even head cols 0:fw, odd fw:2fw
                h0, h1 = 2 * pi, 2 * pi + 1
                nfb = 2 * fw // P
                den = fin.tile([P, nfb], dt.float32, tag="den", name="den")
                nc.sync.dma_start(den[:], ctxs_t[DH:DH + 1, :])
                rec = fin.tile([P, nfb], dt.float32, tag="rec", name="rec")
                nc.vector.reciprocal(rec[:], den[:])
                rr = fin.tile([1, 2 * fw], dt.float32, tag="rr", name="rr")
                nc.sync.dma_start(rr[:], rec[:])
                rrb = fin.tile([DH, 2 * fw], dt.float32, tag="rrb",
                               name="rrb")
                nc.gpsimd.partition_broadcast(rrb[:], rr[:])
                ot = fin.tile([DH, 2 * fw], dt.float32, tag="ot", name="ot")
                nc.vector.tensor_tensor(ot[:], ctxs_t[0:DH, :], rrb[:],
                                        mybir.AluOpType.mult)
                nc.sync.dma_start(
                    out.ap()[h0 * DH:(h0 + 1) * DH, f0:f0 + fw],
                    ot[:, 0:fw])
                nc.sync.dma_start(
                    out.ap()[h1 * DH:(h1 + 1) * DH, f0:f0 + fw],
                    ot[:, fw:2 * fw])

            segs = [(pi, c * FW, FW) for pi in range(NPAIR)
                    for c in range(4)]
            NSEG = len(segs)

            def drain_seg(st):
                # drain + normalize (off the PE critical path)
                pi, f0, fw, cp0, cp1 = st
                ctxs_t = fin.tile([DH + 1, 2 * fw], dt.float32, tag="ctxs",
                                  name="ctxs")
                nc.vector.tensor_copy(ctxs_t[:, 0:fw], cp0[0:DH + 1, :])
                nc.vector.tensor_copy(ctxs_t[:, fw:2 * fw], cp1[0:DH + 1, :])
                normalize(pi, f0, fw, ctxs_t)

            # Software-pipelined across segments: ctx trails scores/exp by
            # ~2 pairs and flows across segment boundaries, so the next
            # segment's scores (and thus exps) are emitted before the
            # previous segment's last ctx -- ACT never waits for a segment
            # flush.  pend entries carry their segment state; a segment's
            # drain+normalize is emitted as soon as its last ctx pops.
            pend = deque()   # (seg_state, tj, et, is_last_of_seg)
            for si, (pi, f0, fw) in enumerate(segs):
                cp0 = ps_c.tile([P, FW], dt.float32, tag="c", name="cp0")
                cp1 = ps_c.tile([P, FW], dt.float32, tag="c", name="cp1")
                st = (pi, f0, fw, cp0, cp1)
                fills = deque(seg_fills[si])
                first = (si == 0)
                # (A Schraudolph bit-trick DVE exp offload for 2 tiles/seg,
                # routed through the proj psum banks, measured speed-neutral
                # while doubling rel err to 6.4e-3 -- dropped.)
                for g in range(NT // 2):
                    t0, t1 = 2 * g, 2 * g + 1
                    sp0 = emit_scores(pi, f0, fw, t0)
                    sp1 = emit_scores(pi, f0, fw, t1)
                    pend.append((st, t0, emit_exp(sp0, fw, t0), False))
                    pend.append((st, t1, emit_exp(sp1, fw, t1),
                                 t1 == NT - 1))
                    # ctx at lag >= 2 exps (sem comfortably clear); seg 0
                    # defers until vp pair-0 exists (group 4+), the backlog
                    # drains at 3/group spilling into following segments.
                    if first and g < 4:
                        nctx = 0
                    elif si == NSEG - 1 and g >= 6:
                        # drain harder near the end so the final flush after
                        # the last exp is as short as possible
                        nctx = min(3, max(0, len(pend) - 1))
                    else:
                        nctx = min(3, max(0, len(pend) - 2))
                    for _ in range(nctx):
                        stj, tj, etj, last = pend.popleft()
                        emit_ctx(stj[3], stj[4], stj[0], stj[2], tj, etj)
                        if last:
                            drain_seg(stj)
                    # fills (128-mode, adjacent to the ctx block); seg0
                    # front-loads (PE in-order: vf0/k0 chunks must precede
                    # the ctx/scores that depend on them)
                    if first:
                        if g < 4:
                            for _ in range(2):
                                if fills:
                                    fills.popleft()()
                    elif fills and g % 2 == 1:
                        fills.popleft()()
                while fills:
                    fills.popleft()()
                if si == NSEG - 1:
                    # flush everything except the final segment's tail
                    while pend:
                        stj, tj, etj, last = pend.popleft()
                        emit_ctx(stj[3], stj[4], stj[0], stj[2], tj, etj)
                        if last and stj is not st:
                            drain_seg(stj)
            # ---------------- tail (last segment normalize) ----------
            if True:
                    pi, f0, fw, cp0, cp1 = st
                    h0, h1 = 2 * pi, 2 * pi + 1
                    ctxs_t = fin.tile([DH + 1, 2 * fw], dt.float32,
                                      tag="ctxs", name="ctxs")
                    nc.vector.tensor_copy(ctxs_t[:, 0:fw], cp0[0:DH + 1, :])
                    nc.vector.tensor_copy(ctxs_t[:, fw:2 * fw],
                                          cp1[0:DH + 1, :])
                    # tail: gather-transpose dens -> reciprocal on 128
                    # lanes -> scatter -> PE ones-broadcast -> multiply ->
                    # store.  (ACT-side 1/x was tried twice: Reciprocal is
                    # hard-blocked in bass, and exp(-ln(x)) forces ~2-4
                    # ACT_TABLE_LOAD+DRAIN pairs because the planner keeps
                    # Exp and Ln in separate table sets.)
                    nfb = 2 * fw // P
                    den = fin.tile([P, nfb], dt.float32, tag="den",
                                   name="den_t")
                    nc.sync.dma_start(den[:], ctxs_t[DH:DH + 1, :])
                    rec = fin.tile([P, nfb], dt.bfloat16, tag="rec",
                                   name="rec_t")
                    with nc.allow_low_precision(
                            reason="bf16 recip feeds PE broadcast; "
                                   "~0.2% rel err is in budget"):
                        nc.vector.reciprocal(rec[:], den[:])
                    rrow = fin.tile([1, 2 * fw], dt.bfloat16, tag="rrow",
                                    name="rrow")
                    nc.sync.dma_start(rrow[:], rec[:])
                    rb = ps_s.tile([DH, 2 * fw], dt.float32, tag="s",
                                   name="rb")
                    nc.tensor.matmul(rb[:, 0:512], ones1[:], rrow[:, 0:512],
                                     start=True, stop=True)
                    nc.tensor.matmul(rb[:, 512:2 * fw], ones1[:],
                                     rrow[:, 512:2 * fw],
                                     start=True, stop=True)
                    ot = fin.tile([DH, 2 * fw], dt.float32, tag="ot",
                                  name="ot_t")
                    nc.vector.tensor_tensor(ot[:], ctxs_t[0:DH, :],
                                            rb[:], mybir.AluOpType.mult)
                    nc.sync.dma_start(
                        out.ap()[h0 * DH:(h0 + 1) * DH, f0:f0 + fw],
                        ot[:, 0:fw])
                    nc.sync.dma_start(
                        out.ap()[h1 * DH:(h1 + 1) * DH, f0:f0 + fw],
                        ot[:, fw:2 * fw])

    nc.compile()
    return nc


def _prep_core_inputs(c, x, Wq, bq, Wk, bk, Wv, bv, mask, ones_mask):
    bf16 = ml_dtypes.bfloat16
    b, hg = c // 2, c % 2
    cols = slice(hg * DL, (hg + 1) * DL)

    xT_ = np.ascontiguousarray(x[b].T.astype(bf16))

    def aug(W, bias):
        w = np.empty((D + 1, DL), dtype=bf16)
        w[:D] = W[:, cols].astype(bf16)
        w[D] = bias[cols].astype(bf16)
        return w

    if ones_mask:
        adder_t = np.zeros((P, NT), dtype=np.float32)
    else:
        add = ((mask[b].astype(np.float32) - 1.0) * 10000.0)
        if TOK_INTERLEAVED:
            adder_t = add.reshape(P, NT).copy()      # [p, ti] = add[p*16+ti]
        else:
            adder_t = add.reshape(NT, P).T.copy()    # [p, ti] = add[ti*128+p]

    return {"xT": xT_, "wq": aug(Wq, bq), "wk": aug(Wk, bk),
            "wv": aug(Wv, bv),
            "adder": np.ascontiguousarray(adder_t, dtype=np.float32)}


def kernel(x, Wq, bq, Wk, bk, Wv, bv, mask, _trace=False):
    from concourse.bass_utils import run_bass_kernel_spmd

    x = np.asarray(x, dtype=np.float32)
    Wq = np.asarray(Wq, dtype=np.float32)
    bq = np.asarray(bq, dtype=np.float32)
    Wk = np.asarray(Wk, dtype=np.float32)
    bk = np.asarray(bk, dtype=np.float32)
    Wv = np.asarray(Wv, dtype=np.float32)
    bv = np.asarray(bv, dtype=np.float32)
    mask = np.asarray(mask)

    with_bias = bool(bq.any() or bk.any() or bv.any())
    ones_mask = bool((mask == 1).all())
    key = ("nc", with_bias, ones_mask)
    if key not in _CACHE:
        _CACHE[key] = _build(with_bias=with_bias, ones_mask=ones_mask)
    nc = _CACHE[key]

    in_maps = [_prep_core_inputs(c, x, Wq, bq, Wk, bk, Wv, bv, mask,
                                 ones_mask)
               for c in range(NCORES)]
    res = run_bass_kernel_spmd(nc, in_maps, core_ids=list(range(NCORES)),
                               trace=_trace)
    if _trace:
        _CACHE["last_result"] = res

    full = np.empty((B, S, D), dtype=np.float32)
    for c in range(NCORES):
        b, hg = c // 2, c % 2
        full[b, :, hg * DL:(hg + 1) * DL] = res.results[c]["out"].T
    return full
